# revision 54
# baseline (speedup 1.0000x reference)
import sys
import os

sys.path.insert(0, "/opt/trn_rl_repo")

import numpy as np
import ml_dtypes

import concourse.bass as bass
import concourse.tile as tile
from concourse import mybir
from concourse.bass_utils import run_bass_kernel_spmd

# ---------------- problem constants (hardcoded) ----------------
B, N, DIM, H, DH, K = 2, 2048, 1024, 8, 64, 32
INNER = H * DH          # 512
NH = 2                  # heads per core
NT = N // 128           # 16 query/key tiles
NQB = N // 512          # 4 query blocks of 512
KJ = K + 1              # 33 mem slots incl null at j=0
KJD = KJ * DH           # 2112
SCALE = DH ** -0.5
NEG = -3.0e38

FP32 = mybir.dt.float32
BF16 = mybir.dt.bfloat16
NPBF16 = ml_dtypes.bfloat16


# ---------------- drain workaround (this walrus rejects multi-wait Drain) ---
def _patched_drain(self, tick_clock, wait_clock):
    nc = self.nc
    drain_inst = nc.sync.drain()
    from concourse.tile import ScopedClock

    wait_clock.add_sem_waits(
        drain_inst.ins, ScopedClock({None: tick_clock.global_clock})
    )
    si = drain_inst.ins.sync_info
    waits = list(si.on_wait)
    if len(waits) > 1:
        drain_inst.ins.sync_info = type(si)(on_wait=waits[:1], on_update=[])
        for w in waits[1:]:
            nop = nc.sync.nop(nofuse=True)
            nop.ins.sync_info = type(si)(on_wait=[w], on_update=[])
    nc.all_engine_barrier()
    popped = nc._tile_sem_poison_stack.pop()
    assert popped is self._sem_poison
    nc.clear_and_free_semaphores(list(self.sems.allocated().values()))
    nc.all_engine_barrier()


tile.TileContext._drain_and_barrier = _patched_drain


# ---- split multi-wait instructions (walrus wait-slot limit) ----
_MAXW = 1
_orig_lower_ordered = tile.TileContext._lower_ordered_insts


def _split_lower(self, ordered):
    n = [0]
    for bbname in list(ordered.keys()):
        insts = ordered[bbname]
        new = []
        for inst in insts:
            try:
                si = inst.sync_info
                waits = list(si.on_wait) if si is not None else []
            except AttributeError:
                waits = []
            if len(waits) > _MAXW:
                keep = waits[-_MAXW:]
                extra = waits[:-_MAXW]
                for i in range(0, len(extra), _MAXW):
                    chunk = extra[i : i + _MAXW]
                    n[0] += 1
                    nop = mybir.InstNoOp(
                        name=f"waitnop-{n[0]}-{inst.name}",
                        sync_info=mybir.SyncInfo(on_wait=chunk, on_update=[]),
                        bass_nofuse=True,
                        engine=inst.engine,
                    )
                    new.append(nop)
                inst.sync_info = mybir.SyncInfo(
                    on_wait=keep, on_update=list(si.on_update)
                )
            new.append(inst)
        ordered[bbname] = new
    print(f"[waitsplit] inserted {n[0]} carrier nops")
    return _orig_lower_ordered(self, ordered)


tile.TileContext._lower_ordered_insts = _split_lower

_PROGRAM = None


def _build_program():
    nc = bass.Bass()
    xT_e = nc.declare_dram_parameter("xT", [DIM, N], BF16, isOutput=False)
    wsb_e = nc.declare_dram_parameter("wsb", [128, 2048], BF16, isOutput=False)
    wo_e = nc.declare_dram_parameter("wo", [NH * DH, DIM], BF16, isOutput=False)
    mkt_e = nc.declare_dram_parameter("mkt", [NH, NT, 64, 4096], BF16, isOutput=False)
    mv_e = nc.declare_dram_parameter("mv", [NH, N, KJD], BF16, isOutput=False)
    nk_e = nc.declare_dram_parameter("nk", [64, 1], BF16, isOutput=False)
    gg_e = nc.declare_dram_parameter("gg", [128, 4], FP32, isOutput=False)
    mask_e = nc.declare_dram_parameter("mask", [128, 128], FP32, isOutput=False)
    iden_e = nc.declare_dram_parameter("iden", [128, 128], BF16, isOutput=False)
    out_e = nc.declare_dram_parameter("out", [N, DIM], BF16, isOutput=True)

    AX = mybir.AxisListType.X
    EXP = mybir.ActivationFunctionType.Exp
    MULT = mybir.AluOpType.mult

    with tile.TileContext(nc) as tc:
        with tc.tile_pool(name="persist", bufs=1) as pp:
            qTh = [pp.tile([64, N], BF16, tag=f"qT{h}", name=f"qT{h}") for h in range(NH)]
            kT = pp.tile([64, N], BF16)
            vone = pp.tile([128, NT * 65], BF16)  # per ki tile: [v | 1]
            nsim_all = pp.tile([128, NH * NT], FP32)  # null-key sims per (h, qi)
            wo_sb = pp.tile([128, DIM], BF16)
            gg_sb = pp.tile([128, 4], FP32)
            mask_sb = pp.tile([128, 128], FP32)
            iden_sb = pp.tile([128, 128], BF16)
            nk_sb = pp.tile([64, 1], BF16)
            nc.sync.dma_start(wo_sb[:], wo_e[:])
            nc.sync.dma_start(gg_sb[:], gg_e[:])
            nc.sync.dma_start(mask_sb[:], mask_e[:])
            nc.sync.dma_start(iden_sb[:], iden_e[:])
            nc.sync.dma_start(nk_sb[:], nk_e[:])

            # ---------------- stage A: projections + transposes ----------------
            with tc.tile_pool(name="stageA", bufs=2) as pa, \
                 tc.tile_pool(name="psA", bufs=2, space="PSUM") as psA:
                w_sb = pa.tile([128, 2048], BF16, tag="w")
                nc.sync.dma_start(w_sb[:], wsb_e[:])
                xt_tiles = []
                for d in range(8):
                    xt = pa.tile([128, N], BF16, tag=f"xt{d}")
                    nc.sync.dma_start(xt[:], xT_e[d * 128 : (d + 1) * 128, :])
                    xt_tiles.append(xt)
                vT = pa.tile([64, N], BF16, tag="vT")
                for nb in range(4):
                    sl = slice(nb * 512, (nb + 1) * 512)
                    targets = [
                        (qTh[0], 0), (qTh[1], 64), (kT, 128), (vT, 192),
                    ]
                    for dst, woff in targets:
                        ps = psA.tile([64, 512], FP32, tag="mm")
                        for d in range(8):
                            nc.tensor.matmul(
                                ps[:],
                                w_sb[:, d * 256 + woff : d * 256 + woff + 64],
                                xt_tiles[d][:, sl],
                                start=(d == 0),
                                stop=(d == 7),
                            )
                        nc.scalar.copy(dst[:, sl], ps[:])
                # null-key sims for all queries: nsim[q] = q . null_k, via PE
                for h in range(NH):
                    psn = psA.tile([128, NT], FP32, tag="psn")
                    for qi in range(NT):
                        nc.tensor.matmul(
                            psn[:, qi : qi + 1],
                            qTh[h][:, qi * 128 : (qi + 1) * 128],
                            nk_sb[:],
                            start=True,
                            stop=True,
                        )
                    nc.scalar.copy(nsim_all[:, h * NT : (h + 1) * NT], psn[:])
                # v_nat (+ ones col)
                for ki in range(NT):
                    tp2 = psA.tile([128, 64], BF16, tag="tp2")
                    nc.tensor.transpose(
                        tp2[:],
                        vT[:, ki * 128 : (ki + 1) * 128],
                        iden_sb[0:64, 0:64],
                    )
                    nc.scalar.copy(vone[:, ki * 65 : ki * 65 + 64], tp2[:])
                    nc.vector.memset(vone[:, ki * 65 + 64 : ki * 65 + 65], 1.0)

            # ---------------- main loop ----------------
            with tc.tile_pool(name="mem", bufs=4) as pm, \
                 tc.tile_pool(name="small", bufs=6) as psm, \
                 tc.tile_pool(name="pts", bufs=36) as ptp, \
                 tc.tile_pool(name="dscrp", bufs=4, space="DRAM") as dscrp, \
                 tc.tile_pool(name="ps_st", bufs=2, space="PSUM") as ps_st, \
                 tc.tile_pool(name="ps_pv", bufs=2, space="PSUM") as ps_pv, \
                 tc.tile_pool(name="ps_sim", bufs=1, space="PSUM") as ps_sim_p, \
                 tc.tile_pool(name="ps_c", bufs=1, space="PSUM") as ps_c:
                for qb in range(NQB):
                    # ---- local causal attention for this 512-query block ----
                    # phase 1: scores + exp for all key tiles, kept in SBUF
                    ptl = {}
                    for h in range(NH):
                        for ki in range(4 * qb + 4):
                            s_rel = ki - 4 * qb
                            qcol0 = max(s_rel, 0) * 128
                            ncols = 512 - qcol0
                            st = ps_st.tile([128, 512], FP32, tag="st")
                            nc.tensor.matmul(
                                st[:, qcol0 : qcol0 + ncols],
                                kT[:, ki * 128 : (ki + 1) * 128],
                                qTh[h][:, qb * 512 + qcol0 : qb * 512 + 512],
                                start=True,
                                stop=True,
                            )
                            if s_rel >= 0:
                                nc.vector.tensor_add(
                                    st[:, qcol0 : qcol0 + 128],
                                    st[:, qcol0 : qcol0 + 128],
                                    mask_sb[:],
                                )
                            pt = ptp.tile([128, 512], BF16, tag="pt", name="pt")
                            nc.scalar.activation(
                                pt[:, qcol0:512], st[:, qcol0:512], EXP, scale=SCALE
                            )
                            ptl[(h, ki)] = pt
                    # ---- phase 2: pv accumulation (one PSUM group per bank
                    # lifetime), mem branch, combine, output proj ----
                    for s in range(4):
                        qi = 4 * qb + s
                        o2 = psm.tile([128, 128], BF16, tag="o2")
                        for h in range(NH):
                            pv = ps_pv.tile([128, 65], FP32, tag="pv")
                            for ki in range(qi + 1):
                                nc.tensor.matmul(
                                    pv[:],
                                    ptl[(h, ki)][:, s * 128 : (s + 1) * 128],
                                    vone[:, ki * 65 : ki * 65 + 65],
                                    start=(ki == 0),
                                    stop=(ki == qi),
                                )
                            # ---- mem sims on PE: 4 col-tiled chunks of 32 queries,
                            # each against its own 1024 stacked keys ----
                            mkt_t = pm.tile([64, 4096], BF16, tag="mkt")
                            nc.sync.dma_start(mkt_t[:], mkt_e[h, qi, :, :])
                            mv_t = pm.tile([128, KJD], BF16, tag="mv")
                            nc.sync.dma_start(mv_t[:], mv_e[h, qi * 128 : (qi + 1) * 128, :])
                            sims = ps_sim_p.tile([128, 1024], FP32, tag="sims")
                            for cc in range(4):
                                for half in range(2):
                                    nc.tensor.matmul(
                                        sims[32 * cc : 32 * cc + 32,
                                             half * 512 : (half + 1) * 512],
                                        qTh[h][:, qi * 128 + 32 * cc : qi * 128 + 32 * cc + 32],
                                        mkt_t[:, cc * 1024 + half * 512 :
                                              cc * 1024 + (half + 1) * 512],
                                        start=True,
                                        stop=True,
                                        tile_position=(0, 32 * cc),
                                    )
                            simsb = psm.tile([128, 1024], BF16, tag="simsb")
                            nc.scalar.copy(simsb[:], sims[:])
                            # keepalive: brief PE activity to hold the HAM
                            # clock-gate warm through the DMA/DVE stretch
                            nc.tensor.ldweights(simsb[:, 0:128])
                            dscr = dscrp.tile([128, 1024], BF16, tag="dscr")
                            nc.scalar.dma_start(dscr[:], simsb[:])
                            sim33 = psm.tile([128, KJ], BF16, tag="sim33")
                            dsel = dscr[:].copy()
                            dsel.ap = mybir.VecI64Pair([[32768, 4], [1056, 32], [1, 32]])
                            nc.sync.dma_start(sim33[:, 1:33], dsel)
                            # null slot from stage-A PE matvec
                            nc.vector.tensor_copy(
                                sim33[:, 0:1], nsim_all[:, h * NT + qi : h * NT + qi + 1]
                            )
                            p33 = psm.tile([128, KJ], BF16, tag="p33")
                            msum = psm.tile([128, 1], FP32, tag="msum")
                            nc.scalar.activation(
                                p33[:], sim33[:], EXP, scale=SCALE, accum_out=msum[:]
                            )
                            nc.tensor.ldweights(p33[:])
                            # weighted values: sum_j p_j * mv_j  (d-major mv)
                            prod2 = pm.tile([128, KJD], BF16, tag="prod2")
                            mv3 = mv_t[:].rearrange("p (d j) -> p d j", j=KJ)
                            p_bc = p33[:].unsqueeze(1).broadcast_to([128, DH, KJ])
                            pr2v = prod2[:].rearrange("p (d j) -> p d j", j=KJ)
                            nc.gpsimd.tensor_mul(pr2v, mv3, p_bc)
                            nc.tensor.ldweights(prod2[:, 0:128])
                            memv = psm.tile([128, DH], FP32, tag="memv")
                            nc.vector.reduce_sum(memv[:], pr2v, axis=AX)
                            # ---- combine: o = pv*g/l + memv*(1-g)/m ----
                            linv = psm.tile([128, 1], FP32, tag="linv")
                            nc.vector.reciprocal(linv[:], pv[:, 64:65])
                            lg = psm.tile([128, 1], FP32, tag="lg")
                            nc.scalar.mul(lg[:], linv[:], gg_sb[:, h : h + 1])
                            minv = psm.tile([128, 1], FP32, tag="minv")
                            nc.vector.reciprocal(minv[:], msum[:])
                            mg = psm.tile([128, 1], FP32, tag="mg")
                            nc.scalar.mul(mg[:], minv[:], gg_sb[:, 2 + h : 3 + h])
                            osl = o2[:, h * 64 : (h + 1) * 64]
                            nc.vector.tensor_scalar(
                                osl, pv[:, 0:64], lg[:], None, op0=MULT,
                            )
                            tmp = psm.tile([128, DH], BF16, tag="tmp")
                            nc.vector.tensor_scalar(
                                tmp[:], memv[:], mg[:], None, op0=MULT
                            )
                            nc.vector.tensor_add(osl, osl, tmp[:])
                        # ---- output projection for this qi ----
                        otp = ps_c.tile([128, 128], BF16, tag="otp")
                        nc.tensor.transpose(otp[:], o2[:], iden_sb[:])
                        ot_sb = psm.tile([128, 128], BF16, tag="otsb")
                        nc.scalar.copy(ot_sb[:], otp[:])
                        for half in range(2):
                            op_ps = ps_c.tile([128, 512], FP32, tag="ops")
                            nc.tensor.matmul(
                                op_ps[:],
                                ot_sb[:],
                                wo_sb[:, half * 512 : (half + 1) * 512],
                                start=True,
                                stop=True,
                            )
                            out_sb = psm.tile([128, 512], BF16, tag="outsb")
                            nc.scalar.copy(out_sb[:], op_ps[:])
                            nc.sync.dma_start(
                                out_e[qi * 128 : (qi + 1) * 128,
                                      half * 512 : (half + 1) * 512],
                                out_sb[:],
                            )
    return nc


def _get_program():
    global _PROGRAM
    if _PROGRAM is None:
        _PROGRAM = _build_program()
    return _PROGRAM


def kernel(x, Wq, Wkv, Wo, bo, null_k, null_v, gate, mem_kv, mem_mask):
    x = np.asarray(x, dtype=np.float32)
    Wq = np.asarray(Wq, dtype=np.float32)
    Wkv = np.asarray(Wkv, dtype=np.float32)
    Wo = np.asarray(Wo, dtype=np.float32)
    bo = np.asarray(bo, dtype=np.float32)
    null_k = np.asarray(null_k, dtype=np.float32)
    null_v = np.asarray(null_v, dtype=np.float32)
    gate = np.asarray(gate, dtype=np.float32)
    mem_kv = np.asarray(mem_kv, dtype=np.float32)

    nc = _get_program()
    g = 1.0 / (1.0 + np.exp(-gate.reshape(H)))  # sigmoid, per head
    mem6 = mem_kv.reshape(B, H, N, K, 2, DH)

    # mem keys transposed for PE: [B, H, NT, 64, 4096], col = m*32 + j
    mkt_all = np.ascontiguousarray(
        mem6[..., 0, :]
        .reshape(B, H, NT, 128, K, DH)
        .transpose(0, 1, 2, 5, 3, 4)
        .reshape(B, H, NT, DH, 128 * K)
    ).astype(NPBF16)
    # mem values, d-major with null slot at j=0: [B, H, N, 64, 33]
    mv_all = np.empty((B, H, N, DH, KJ), dtype=NPBF16)
    mv_all[..., 0] = null_v.astype(NPBF16)
    mv_all[..., 1:] = np.swapaxes(mem6[..., 1, :], -1, -2).astype(NPBF16)

    xb = x.astype(NPBF16)
    Wqb = Wq.astype(NPBF16)
    Wkvb = Wkv.astype(NPBF16)
    Wob = Wo.astype(NPBF16)

    iden = np.eye(128, dtype=NPBF16)
    nk_col = null_k.reshape(DH, 1).astype(NPBF16)
    maskT = np.where(
        np.arange(128)[:, None] <= np.arange(128)[None, :], 0.0, NEG
    ).astype(np.float32)

    in_maps = []
    for c in range(8):
        b, hg = c // 4, c % 4
        h0 = hg * NH
        xT = np.ascontiguousarray(xb[b].T)
        wsb = np.empty((128, 2048), dtype=NPBF16)
        for d in range(8):
            wsb[:, d * 256 : d * 256 + 128] = Wqb[
                d * 128 : (d + 1) * 128, h0 * DH : (h0 + NH) * DH
            ]
            wsb[:, d * 256 + 128 : d * 256 + 256] = Wkvb[d * 128 : (d + 1) * 128, :]
        wo_c = np.ascontiguousarray(Wob[h0 * DH : (h0 + NH) * DH, :])
        mkt_c = mkt_all[b, h0 : h0 + NH]
        mv_c = mv_all[b, h0 : h0 + NH].reshape(NH, N, KJD)
        gg = np.zeros((128, 4), dtype=np.float32)
        gg[:, 0] = g[h0]
        gg[:, 1] = g[h0 + 1]
        gg[:, 2] = 1.0 - g[h0]
        gg[:, 3] = 1.0 - g[h0 + 1]
        in_maps.append(
            dict(
                xT=xT, wsb=wsb, wo=wo_c, mkt=mkt_c, mv=mv_c,
                nk=nk_col, gg=gg, mask=maskT, iden=iden,
            )
        )

    global _last_in_maps
    _last_in_maps = in_maps
    res = run_bass_kernel_spmd(nc, in_maps, list(range(8)))
    out = np.zeros((B, N, DIM), dtype=np.float32)
    for c in range(8):
        out[c // 4] += res.results[c]["out"].astype(np.float32)
    out += bo[None, None, :]
    return out


# revision 57
# speedup vs baseline: 1.0339x; 1.0339x over previous
import sys
import os

sys.path.insert(0, "/opt/trn_rl_repo")

import numpy as np
import ml_dtypes

import concourse.bass as bass
import concourse.tile as tile
from concourse import mybir
from concourse.bass_utils import run_bass_kernel_spmd

# ---------------- problem constants (hardcoded) ----------------
B, N, DIM, H, DH, K = 2, 2048, 1024, 8, 64, 32
INNER = H * DH          # 512
NH = 2                  # heads per core
NT = N // 128           # 16 query/key tiles
NQB = N // 512          # 4 query blocks of 512
KJ = K + 1              # 33 mem slots incl null at j=0
KJD = KJ * DH           # 2112
SCALE = DH ** -0.5
NEG = -3.0e38

FP32 = mybir.dt.float32
BF16 = mybir.dt.bfloat16
NPBF16 = ml_dtypes.bfloat16


# ---------------- drain workaround (this walrus rejects multi-wait Drain) ---
def _patched_drain(self, tick_clock, wait_clock):
    nc = self.nc
    drain_inst = nc.sync.drain()
    from concourse.tile import ScopedClock

    wait_clock.add_sem_waits(
        drain_inst.ins, ScopedClock({None: tick_clock.global_clock})
    )
    si = drain_inst.ins.sync_info
    waits = list(si.on_wait)
    if len(waits) > 1:
        drain_inst.ins.sync_info = type(si)(on_wait=waits[:1], on_update=[])
        for w in waits[1:]:
            nop = nc.sync.nop(nofuse=True)
            nop.ins.sync_info = type(si)(on_wait=[w], on_update=[])
    nc.all_engine_barrier()
    popped = nc._tile_sem_poison_stack.pop()
    assert popped is self._sem_poison
    nc.clear_and_free_semaphores(list(self.sems.allocated().values()))
    nc.all_engine_barrier()


tile.TileContext._drain_and_barrier = _patched_drain


# ---- split multi-wait instructions (walrus wait-slot limit) ----
_MAXW = 1
_orig_lower_ordered = tile.TileContext._lower_ordered_insts


def _split_lower(self, ordered):
    n = [0]
    for bbname in list(ordered.keys()):
        insts = ordered[bbname]
        new = []
        for inst in insts:
            try:
                si = inst.sync_info
                waits = list(si.on_wait) if si is not None else []
            except AttributeError:
                waits = []
            if len(waits) > _MAXW:
                keep = waits[-_MAXW:]
                extra = waits[:-_MAXW]
                for i in range(0, len(extra), _MAXW):
                    chunk = extra[i : i + _MAXW]
                    n[0] += 1
                    nop = mybir.InstNoOp(
                        name=f"waitnop-{n[0]}-{inst.name}",
                        sync_info=mybir.SyncInfo(on_wait=chunk, on_update=[]),
                        bass_nofuse=True,
                        engine=inst.engine,
                    )
                    new.append(nop)
                inst.sync_info = mybir.SyncInfo(
                    on_wait=keep, on_update=list(si.on_update)
                )
            new.append(inst)
        ordered[bbname] = new
    print(f"[waitsplit] inserted {n[0]} carrier nops")
    return _orig_lower_ordered(self, ordered)


tile.TileContext._lower_ordered_insts = _split_lower

_PROGRAM = None


def _build_program():
    nc = bass.Bass()
    xT_e = nc.declare_dram_parameter("xT", [DIM, N], BF16, isOutput=False)
    wsb_e = nc.declare_dram_parameter("wsb", [128, 2048], BF16, isOutput=False)
    wo_e = nc.declare_dram_parameter("wo", [NH * DH, DIM], BF16, isOutput=False)
    mkt_e = nc.declare_dram_parameter("mkt", [NH, NT, 64, 4096], BF16, isOutput=False)
    mv_e = nc.declare_dram_parameter("mv", [NH, N, KJD], BF16, isOutput=False)
    nk_e = nc.declare_dram_parameter("nk", [64, 1], BF16, isOutput=False)
    gg_e = nc.declare_dram_parameter("gg", [128, 4], FP32, isOutput=False)
    mask_e = nc.declare_dram_parameter("mask", [128, 128], FP32, isOutput=False)
    iden_e = nc.declare_dram_parameter("iden", [128, 128], BF16, isOutput=False)
    out_e = nc.declare_dram_parameter("out", [N, DIM], BF16, isOutput=True)

    AX = mybir.AxisListType.X
    EXP = mybir.ActivationFunctionType.Exp
    MULT = mybir.AluOpType.mult

    with tile.TileContext(nc) as tc:
        with tc.tile_pool(name="persist", bufs=1) as pp:
            qTh = [pp.tile([64, N], BF16, tag=f"qT{h}", name=f"qT{h}") for h in range(NH)]
            kT = pp.tile([64, N], BF16)
            vone = pp.tile([128, NT * 65], BF16)  # per ki tile: [v | 1]
            nsim_all = pp.tile([128, NH * NT], FP32)  # null-key sims per (h, qi)
            wo_sb = pp.tile([128, DIM], BF16)
            gg_sb = pp.tile([128, 4], FP32)
            mask_sb = pp.tile([128, 128], FP32)
            iden_sb = pp.tile([128, 128], BF16)
            nk_sb = pp.tile([64, 1], BF16)
            nc.sync.dma_start(wo_sb[:], wo_e[:])
            nc.sync.dma_start(gg_sb[:], gg_e[:])
            nc.sync.dma_start(mask_sb[:], mask_e[:])
            nc.sync.dma_start(iden_sb[:], iden_e[:])
            nc.sync.dma_start(nk_sb[:], nk_e[:])

            # ---------------- stage A: projections + transposes ----------------
            with tc.tile_pool(name="stageA", bufs=2) as pa, \
                 tc.tile_pool(name="psA", bufs=2, space="PSUM") as psA:
                w_sb = pa.tile([128, 2048], BF16, tag="w")
                nc.sync.dma_start(w_sb[:], wsb_e[:])
                xt_tiles = []
                for d in range(8):
                    xt = pa.tile([128, N], BF16, tag=f"xt{d}")
                    nc.sync.dma_start(xt[:], xT_e[d * 128 : (d + 1) * 128, :])
                    xt_tiles.append(xt)
                vT = pa.tile([64, N], BF16, tag="vT")
                for nb in range(4):
                    sl = slice(nb * 512, (nb + 1) * 512)
                    targets = [
                        (qTh[0], 0), (qTh[1], 64), (kT, 128), (vT, 192),
                    ]
                    for dst, woff in targets:
                        ps = psA.tile([64, 512], FP32, tag="mm")
                        for d in range(8):
                            nc.tensor.matmul(
                                ps[:],
                                w_sb[:, d * 256 + woff : d * 256 + woff + 64],
                                xt_tiles[d][:, sl],
                                start=(d == 0),
                                stop=(d == 7),
                            )
                        nc.scalar.copy(dst[:, sl], ps[:])
                # null-key sims for all queries: nsim[q] = q . null_k, via PE
                for h in range(NH):
                    psn = psA.tile([128, NT], FP32, tag="psn")
                    for qi in range(NT):
                        nc.tensor.matmul(
                            psn[:, qi : qi + 1],
                            qTh[h][:, qi * 128 : (qi + 1) * 128],
                            nk_sb[:],
                            start=True,
                            stop=True,
                        )
                    nc.scalar.copy(nsim_all[:, h * NT : (h + 1) * NT], psn[:])
                # v_nat (+ ones col)
                for ki in range(NT):
                    tp2 = psA.tile([128, 64], BF16, tag="tp2")
                    nc.tensor.transpose(
                        tp2[:],
                        vT[:, ki * 128 : (ki + 1) * 128],
                        iden_sb[0:64, 0:64],
                    )
                    nc.scalar.copy(vone[:, ki * 65 : ki * 65 + 64], tp2[:])
                    nc.vector.memset(vone[:, ki * 65 + 64 : ki * 65 + 65], 1.0)

            # ---------------- main loop ----------------
            with tc.tile_pool(name="mem", bufs=4) as pm, \
                 tc.tile_pool(name="small", bufs=6) as psm, \
                 tc.tile_pool(name="pts", bufs=36) as ptp, \
                 tc.tile_pool(name="dscrp", bufs=4, space="DRAM") as dscrp, \
                 tc.tile_pool(name="ps_st", bufs=2, space="PSUM") as ps_st, \
                 tc.tile_pool(name="ps_pv", bufs=2, space="PSUM") as ps_pv, \
                 tc.tile_pool(name="ps_sim", bufs=1, space="PSUM") as ps_sim_p, \
                 tc.tile_pool(name="ps_c", bufs=1, space="PSUM") as ps_c:
                for qb in range(NQB):
                    # ---- local causal attention for this 512-query block ----
                    # phase 1: scores + exp for all key tiles, kept in SBUF
                    ptl = {}
                    for h in range(NH):
                        for ki in range(4 * qb + 4):
                            s_rel = ki - 4 * qb
                            qcol0 = max(s_rel, 0) * 128
                            ncols = 512 - qcol0
                            st = ps_st.tile([128, 512], FP32, tag="st")
                            nc.tensor.matmul(
                                st[:, qcol0 : qcol0 + ncols],
                                kT[:, ki * 128 : (ki + 1) * 128],
                                qTh[h][:, qb * 512 + qcol0 : qb * 512 + 512],
                                start=True,
                                stop=True,
                            )
                            if s_rel >= 0:
                                nc.vector.tensor_add(
                                    st[:, qcol0 : qcol0 + 128],
                                    st[:, qcol0 : qcol0 + 128],
                                    mask_sb[:],
                                )
                            pt = ptp.tile([128, 512], BF16, tag="pt", name="pt")
                            nc.scalar.activation(
                                pt[:, qcol0:512], st[:, qcol0:512], EXP, scale=SCALE
                            )
                            ptl[(h, ki)] = pt
                    # ---- phase 2: pv accumulation (one PSUM group per bank
                    # lifetime), mem branch, combine, output proj ----
                    for s in range(4):
                        qi = 4 * qb + s
                        o2 = psm.tile([128, 128], BF16, tag="o2")
                        for h in range(NH):
                            pv = ps_pv.tile([128, 65], FP32, tag="pv")
                            for ki in range(qi + 1):
                                nc.tensor.matmul(
                                    pv[:],
                                    ptl[(h, ki)][:, s * 128 : (s + 1) * 128],
                                    vone[:, ki * 65 : ki * 65 + 65],
                                    start=(ki == 0),
                                    stop=(ki == qi),
                                )
                            # ---- mem sims on PE: 4 col-tiled chunks of 32 queries,
                            # each against its own 1024 stacked keys ----
                            mkt_t = pm.tile([64, 4096], BF16, tag="mkt")
                            nc.sync.dma_start(mkt_t[:], mkt_e[h, qi, :, :])
                            mv_t = pm.tile([128, KJD], BF16, tag="mv")
                            nc.sync.dma_start(mv_t[:], mv_e[h, qi * 128 : (qi + 1) * 128, :])
                            sims = ps_sim_p.tile([128, 1024], FP32, tag="sims")
                            for cc in range(4):
                                for half in range(2):
                                    nc.tensor.matmul(
                                        sims[32 * cc : 32 * cc + 32,
                                             half * 512 : (half + 1) * 512],
                                        qTh[h][:, qi * 128 + 32 * cc : qi * 128 + 32 * cc + 32],
                                        mkt_t[:, cc * 1024 + half * 512 :
                                              cc * 1024 + (half + 1) * 512],
                                        start=True,
                                        stop=True,
                                        tile_position=(0, 32 * cc),
                                    )
                            simsb = psm.tile([128, 1024], BF16, tag="simsb")
                            nc.scalar.copy(simsb[:], sims[:])
                            dscr = dscrp.tile([128, 1024], BF16, tag="dscr")
                            nc.scalar.dma_start(dscr[:], simsb[:])
                            sim33 = psm.tile([128, KJ], BF16, tag="sim33")
                            dsel = dscr[:].copy()
                            dsel.ap = mybir.VecI64Pair([[32768, 4], [1056, 32], [1, 32]])
                            nc.sync.dma_start(sim33[:, 1:33], dsel)
                            # null slot from stage-A PE matvec
                            nc.vector.tensor_copy(
                                sim33[:, 0:1], nsim_all[:, h * NT + qi : h * NT + qi + 1]
                            )
                            p33 = psm.tile([128, KJ], BF16, tag="p33")
                            msum = psm.tile([128, 1], FP32, tag="msum")
                            nc.scalar.activation(
                                p33[:], sim33[:], EXP, scale=SCALE, accum_out=msum[:]
                            )
                            # weighted values: sum_j p_j * mv_j  (d-major mv)
                            prod2 = pm.tile([128, KJD], BF16, tag="prod2")
                            mv3 = mv_t[:].rearrange("p (d j) -> p d j", j=KJ)
                            p_bc = p33[:].unsqueeze(1).broadcast_to([128, DH, KJ])
                            pr2v = prod2[:].rearrange("p (d j) -> p d j", j=KJ)
                            nc.gpsimd.tensor_mul(pr2v, mv3, p_bc)
                            memv = psm.tile([128, DH], FP32, tag="memv")
                            nc.vector.reduce_sum(memv[:], pr2v, axis=AX)
                            # ---- combine: o = pv*g/l + memv*(1-g)/m ----
                            linv = psm.tile([128, 1], FP32, tag="linv")
                            nc.vector.reciprocal(linv[:], pv[:, 64:65])
                            lg = psm.tile([128, 1], FP32, tag="lg")
                            nc.scalar.mul(lg[:], linv[:], gg_sb[:, h : h + 1])
                            minv = psm.tile([128, 1], FP32, tag="minv")
                            nc.vector.reciprocal(minv[:], msum[:])
                            mg = psm.tile([128, 1], FP32, tag="mg")
                            nc.scalar.mul(mg[:], minv[:], gg_sb[:, 2 + h : 3 + h])
                            osl = o2[:, h * 64 : (h + 1) * 64]
                            nc.vector.tensor_scalar(
                                osl, pv[:, 0:64], lg[:], None, op0=MULT,
                            )
                            tmp = psm.tile([128, DH], BF16, tag="tmp")
                            nc.vector.tensor_scalar(
                                tmp[:], memv[:], mg[:], None, op0=MULT
                            )
                            nc.vector.tensor_add(osl, osl, tmp[:])
                        # ---- output projection for this qi ----
                        otp = ps_c.tile([128, 128], BF16, tag="otp")
                        nc.tensor.transpose(otp[:], o2[:], iden_sb[:])
                        ot_sb = psm.tile([128, 128], BF16, tag="otsb")
                        nc.scalar.copy(ot_sb[:], otp[:])
                        for half in range(2):
                            op_ps = ps_c.tile([128, 512], FP32, tag="ops")
                            nc.tensor.matmul(
                                op_ps[:],
                                ot_sb[:],
                                wo_sb[:, half * 512 : (half + 1) * 512],
                                start=True,
                                stop=True,
                            )
                            out_sb = psm.tile([128, 512], BF16, tag="outsb")
                            nc.scalar.copy(out_sb[:], op_ps[:])
                            nc.sync.dma_start(
                                out_e[qi * 128 : (qi + 1) * 128,
                                      half * 512 : (half + 1) * 512],
                                out_sb[:],
                            )
    return nc


def _get_program():
    global _PROGRAM
    if _PROGRAM is None:
        _PROGRAM = _build_program()
    return _PROGRAM


def kernel(x, Wq, Wkv, Wo, bo, null_k, null_v, gate, mem_kv, mem_mask):
    x = np.asarray(x, dtype=np.float32)
    Wq = np.asarray(Wq, dtype=np.float32)
    Wkv = np.asarray(Wkv, dtype=np.float32)
    Wo = np.asarray(Wo, dtype=np.float32)
    bo = np.asarray(bo, dtype=np.float32)
    null_k = np.asarray(null_k, dtype=np.float32)
    null_v = np.asarray(null_v, dtype=np.float32)
    gate = np.asarray(gate, dtype=np.float32)
    mem_kv = np.asarray(mem_kv, dtype=np.float32)

    nc = _get_program()
    g = 1.0 / (1.0 + np.exp(-gate.reshape(H)))  # sigmoid, per head
    mem6 = mem_kv.reshape(B, H, N, K, 2, DH)

    # mem keys transposed for PE: [B, H, NT, 64, 4096], col = m*32 + j
    mkt_all = np.ascontiguousarray(
        mem6[..., 0, :]
        .reshape(B, H, NT, 128, K, DH)
        .transpose(0, 1, 2, 5, 3, 4)
        .reshape(B, H, NT, DH, 128 * K)
    ).astype(NPBF16)
    # mem values, d-major with null slot at j=0: [B, H, N, 64, 33]
    mv_all = np.empty((B, H, N, DH, KJ), dtype=NPBF16)
    mv_all[..., 0] = null_v.astype(NPBF16)
    mv_all[..., 1:] = np.swapaxes(mem6[..., 1, :], -1, -2).astype(NPBF16)

    xb = x.astype(NPBF16)
    Wqb = Wq.astype(NPBF16)
    Wkvb = Wkv.astype(NPBF16)
    Wob = Wo.astype(NPBF16)

    iden = np.eye(128, dtype=NPBF16)
    nk_col = null_k.reshape(DH, 1).astype(NPBF16)
    maskT = np.where(
        np.arange(128)[:, None] <= np.arange(128)[None, :], 0.0, NEG
    ).astype(np.float32)

    in_maps = []
    for c in range(8):
        b, hg = c // 4, c % 4
        h0 = hg * NH
        xT = np.ascontiguousarray(xb[b].T)
        wsb = np.empty((128, 2048), dtype=NPBF16)
        for d in range(8):
            wsb[:, d * 256 : d * 256 + 128] = Wqb[
                d * 128 : (d + 1) * 128, h0 * DH : (h0 + NH) * DH
            ]
            wsb[:, d * 256 + 128 : d * 256 + 256] = Wkvb[d * 128 : (d + 1) * 128, :]
        wo_c = np.ascontiguousarray(Wob[h0 * DH : (h0 + NH) * DH, :])
        mkt_c = mkt_all[b, h0 : h0 + NH]
        mv_c = mv_all[b, h0 : h0 + NH].reshape(NH, N, KJD)
        gg = np.zeros((128, 4), dtype=np.float32)
        gg[:, 0] = g[h0]
        gg[:, 1] = g[h0 + 1]
        gg[:, 2] = 1.0 - g[h0]
        gg[:, 3] = 1.0 - g[h0 + 1]
        in_maps.append(
            dict(
                xT=xT, wsb=wsb, wo=wo_c, mkt=mkt_c, mv=mv_c,
                nk=nk_col, gg=gg, mask=maskT, iden=iden,
            )
        )

    global _last_in_maps
    _last_in_maps = in_maps
    res = run_bass_kernel_spmd(nc, in_maps, list(range(8)))
    out = np.zeros((B, N, DIM), dtype=np.float32)
    for c in range(8):
        out[c // 4] += res.results[c]["out"].astype(np.float32)
    out += bo[None, None, :]
    return out


# revision 59
# speedup vs baseline: 1.0502x; 1.0158x over previous
import sys
import os

sys.path.insert(0, "/opt/trn_rl_repo")

import numpy as np
import ml_dtypes

import concourse.bass as bass
import concourse.tile as tile
from concourse import mybir
from concourse.bass_utils import run_bass_kernel_spmd

# ---------------- problem constants (hardcoded) ----------------
B, N, DIM, H, DH, K = 2, 2048, 1024, 8, 64, 32
INNER = H * DH          # 512
NH = 2                  # heads per core
NT = N // 128           # 16 query/key tiles
NQB = N // 512          # 4 query blocks of 512
KJ = K + 1              # 33 mem slots incl null at j=0
KJD = KJ * DH           # 2112
SCALE = DH ** -0.5
NEG = -3.0e38

FP32 = mybir.dt.float32
BF16 = mybir.dt.bfloat16
NPBF16 = ml_dtypes.bfloat16


# ---------------- drain workaround (this walrus rejects multi-wait Drain) ---
def _patched_drain(self, tick_clock, wait_clock):
    nc = self.nc
    drain_inst = nc.sync.drain()
    from concourse.tile import ScopedClock

    wait_clock.add_sem_waits(
        drain_inst.ins, ScopedClock({None: tick_clock.global_clock})
    )
    si = drain_inst.ins.sync_info
    waits = list(si.on_wait)
    if len(waits) > 1:
        drain_inst.ins.sync_info = type(si)(on_wait=waits[:1], on_update=[])
        for w in waits[1:]:
            nop = nc.sync.nop(nofuse=True)
            nop.ins.sync_info = type(si)(on_wait=[w], on_update=[])
    nc.all_engine_barrier()
    popped = nc._tile_sem_poison_stack.pop()
    assert popped is self._sem_poison
    nc.clear_and_free_semaphores(list(self.sems.allocated().values()))
    nc.all_engine_barrier()


tile.TileContext._drain_and_barrier = _patched_drain


# ---- split multi-wait instructions (walrus wait-slot limit) ----
_MAXW = 1
_orig_lower_ordered = tile.TileContext._lower_ordered_insts


def _split_lower(self, ordered):
    n = [0]
    for bbname in list(ordered.keys()):
        insts = ordered[bbname]
        new = []
        for inst in insts:
            try:
                si = inst.sync_info
                waits = list(si.on_wait) if si is not None else []
            except AttributeError:
                waits = []
            if len(waits) > _MAXW:
                keep = waits[-_MAXW:]
                extra = waits[:-_MAXW]
                for i in range(0, len(extra), _MAXW):
                    chunk = extra[i : i + _MAXW]
                    n[0] += 1
                    nop = mybir.InstNoOp(
                        name=f"waitnop-{n[0]}-{inst.name}",
                        sync_info=mybir.SyncInfo(on_wait=chunk, on_update=[]),
                        bass_nofuse=True,
                        engine=inst.engine,
                    )
                    new.append(nop)
                inst.sync_info = mybir.SyncInfo(
                    on_wait=keep, on_update=list(si.on_update)
                )
            new.append(inst)
        ordered[bbname] = new
    print(f"[waitsplit] inserted {n[0]} carrier nops")
    return _orig_lower_ordered(self, ordered)


tile.TileContext._lower_ordered_insts = _split_lower

_PROGRAM = None


def _build_program():
    nc = bass.Bass()
    xT_e = nc.declare_dram_parameter("xT", [DIM, N], BF16, isOutput=False)
    wsb_e = nc.declare_dram_parameter("wsb", [128, 2048], BF16, isOutput=False)
    wo_e = nc.declare_dram_parameter("wo", [NH * DH, DIM], BF16, isOutput=False)
    mkt_e = nc.declare_dram_parameter("mkt", [NH, NT, 64, 4096], BF16, isOutput=False)
    mv_e = nc.declare_dram_parameter("mv", [NH, N, KJD], BF16, isOutput=False)
    nk_e = nc.declare_dram_parameter("nk", [64, 1], BF16, isOutput=False)
    gg_e = nc.declare_dram_parameter("gg", [128, 4], FP32, isOutput=False)
    mask_e = nc.declare_dram_parameter("mask", [128, 128], FP32, isOutput=False)
    iden_e = nc.declare_dram_parameter("iden", [128, 128], BF16, isOutput=False)
    out_e = nc.declare_dram_parameter("out", [N, DIM], BF16, isOutput=True)

    AX = mybir.AxisListType.X
    EXP = mybir.ActivationFunctionType.Exp
    MULT = mybir.AluOpType.mult

    with tile.TileContext(nc) as tc:
        with tc.tile_pool(name="persist", bufs=1) as pp:
            qTh = [pp.tile([64, N], BF16, tag=f"qT{h}", name=f"qT{h}") for h in range(NH)]
            kT = pp.tile([64, N], BF16)
            vone = pp.tile([128, NT * 65], BF16)  # per ki tile: [v | 1]
            nsim_all = pp.tile([128, NH * NT], FP32)  # null-key sims per (h, qi)
            wo_sb = pp.tile([128, DIM], BF16)
            gg_sb = pp.tile([128, 4], FP32)
            mask_sb = pp.tile([128, 128], FP32)
            iden_sb = pp.tile([128, 128], BF16)
            nk_sb = pp.tile([64, 1], BF16)
            nc.sync.dma_start(wo_sb[:], wo_e[:])
            nc.sync.dma_start(gg_sb[:], gg_e[:])
            nc.sync.dma_start(mask_sb[:], mask_e[:])
            nc.sync.dma_start(iden_sb[:], iden_e[:])
            nc.sync.dma_start(nk_sb[:], nk_e[:])

            # ---------------- stage A: projections + transposes ----------------
            with tc.tile_pool(name="stageA", bufs=2) as pa, \
                 tc.tile_pool(name="psA", bufs=2, space="PSUM") as psA:
                w_sb = pa.tile([128, 2048], BF16, tag="w")
                nc.sync.dma_start(w_sb[:], wsb_e[:])
                xt_tiles = []
                for d in range(8):
                    xt = pa.tile([128, N], BF16, tag=f"xt{d}")
                    nc.sync.dma_start(xt[:], xT_e[d * 128 : (d + 1) * 128, :])
                    xt_tiles.append(xt)
                vT = pa.tile([64, N], BF16, tag="vT")
                for nb in range(4):
                    sl = slice(nb * 512, (nb + 1) * 512)
                    targets = [
                        (qTh[0], 0), (qTh[1], 64), (kT, 128), (vT, 192),
                    ]
                    for dst, woff in targets:
                        ps = psA.tile([64, 512], FP32, tag="mm")
                        for d in range(8):
                            nc.tensor.matmul(
                                ps[:],
                                w_sb[:, d * 256 + woff : d * 256 + woff + 64],
                                xt_tiles[d][:, sl],
                                start=(d == 0),
                                stop=(d == 7),
                            )
                        nc.scalar.copy(dst[:, sl], ps[:])
                # null-key sims for all queries: nsim[q] = q . null_k, via PE
                for h in range(NH):
                    psn = psA.tile([128, NT], FP32, tag="psn")
                    for qi in range(NT):
                        nc.tensor.matmul(
                            psn[:, qi : qi + 1],
                            qTh[h][:, qi * 128 : (qi + 1) * 128],
                            nk_sb[:],
                            start=True,
                            stop=True,
                        )
                    nc.scalar.copy(nsim_all[:, h * NT : (h + 1) * NT], psn[:])
                # v_nat (+ ones col)
                for ki in range(NT):
                    tp2 = psA.tile([128, 64], BF16, tag="tp2")
                    nc.tensor.transpose(
                        tp2[:],
                        vT[:, ki * 128 : (ki + 1) * 128],
                        iden_sb[0:64, 0:64],
                    )
                    nc.scalar.copy(vone[:, ki * 65 : ki * 65 + 64], tp2[:])
                    nc.vector.memset(vone[:, ki * 65 + 64 : ki * 65 + 65], 1.0)

            # ---------------- main loop ----------------
            with tc.tile_pool(name="mem", bufs=4) as pm, \
                 tc.tile_pool(name="small", bufs=6) as psm, \
                 tc.tile_pool(name="pts", bufs=36) as ptp, \
                 tc.tile_pool(name="dscrp", bufs=4, space="DRAM") as dscrp, \
                 tc.tile_pool(name="ps_st", bufs=2, space="PSUM") as ps_st, \
                 tc.tile_pool(name="ps_pv", bufs=2, space="PSUM") as ps_pv, \
                 tc.tile_pool(name="ps_sim", bufs=2, space="PSUM") as ps_sim_p, \
                 tc.tile_pool(name="ps_c", bufs=1, space="PSUM") as ps_c:
                for qb in range(NQB):
                    # ---- local causal attention for this 512-query block ----
                    # phase 1: scores + exp for all key tiles, kept in SBUF
                    ptl = {}
                    for h in range(NH):
                        for ki in range(4 * qb + 4):
                            s_rel = ki - 4 * qb
                            qcol0 = max(s_rel, 0) * 128
                            ncols = 512 - qcol0
                            st = ps_st.tile([128, 512], FP32, tag="st")
                            nc.tensor.matmul(
                                st[:, qcol0 : qcol0 + ncols],
                                kT[:, ki * 128 : (ki + 1) * 128],
                                qTh[h][:, qb * 512 + qcol0 : qb * 512 + 512],
                                start=True,
                                stop=True,
                            )
                            if s_rel >= 0:
                                nc.vector.tensor_add(
                                    st[:, qcol0 : qcol0 + 128],
                                    st[:, qcol0 : qcol0 + 128],
                                    mask_sb[:],
                                )
                            pt = ptp.tile([128, 512], BF16, tag="pt", name="pt")
                            nc.scalar.activation(
                                pt[:, qcol0:512], st[:, qcol0:512], EXP, scale=SCALE
                            )
                            ptl[(h, ki)] = pt
                    # ---- phase 2: pv accumulation (one PSUM group per bank
                    # lifetime), mem branch, combine, output proj ----
                    for s in range(4):
                        qi = 4 * qb + s
                        o2 = psm.tile([128, 128], BF16, tag="o2")
                        for h in range(NH):
                            pv = ps_pv.tile([128, 65], FP32, tag="pv")
                            for ki in range(qi + 1):
                                nc.tensor.matmul(
                                    pv[:],
                                    ptl[(h, ki)][:, s * 128 : (s + 1) * 128],
                                    vone[:, ki * 65 : ki * 65 + 65],
                                    start=(ki == 0),
                                    stop=(ki == qi),
                                )
                            # ---- mem sims on PE: 4 col-tiled chunks of 32 queries,
                            # each against its own 1024 stacked keys ----
                            mkt_t = pm.tile([64, 4096], BF16, tag="mkt")
                            nc.sync.dma_start(mkt_t[:], mkt_e[h, qi, :, :])
                            mv_t = pm.tile([128, KJD], BF16, tag="mv")
                            nc.sync.dma_start(mv_t[:], mv_e[h, qi * 128 : (qi + 1) * 128, :])
                            simsb = psm.tile([128, 1024], BF16, tag="simsb")
                            for half in range(2):
                                sims = ps_sim_p.tile([128, 512], FP32, tag="sims")
                                for cc in range(4):
                                    nc.tensor.matmul(
                                        sims[32 * cc : 32 * cc + 32, :],
                                        qTh[h][:, qi * 128 + 32 * cc : qi * 128 + 32 * cc + 32],
                                        mkt_t[:, cc * 1024 + half * 512 :
                                              cc * 1024 + (half + 1) * 512],
                                        start=True,
                                        stop=True,
                                        tile_position=(0, 32 * cc),
                                    )
                                nc.scalar.copy(
                                    simsb[:, half * 512 : (half + 1) * 512], sims[:]
                                )
                            dscr = dscrp.tile([128, 1024], BF16, tag="dscr")
                            nc.scalar.dma_start(dscr[:], simsb[:])
                            sim33 = psm.tile([128, KJ], BF16, tag="sim33")
                            dsel = dscr[:].copy()
                            dsel.ap = mybir.VecI64Pair([[32768, 4], [1056, 32], [1, 32]])
                            nc.sync.dma_start(sim33[:, 1:33], dsel)
                            # null slot from stage-A PE matvec
                            nc.vector.tensor_copy(
                                sim33[:, 0:1], nsim_all[:, h * NT + qi : h * NT + qi + 1]
                            )
                            p33 = psm.tile([128, KJ], BF16, tag="p33")
                            msum = psm.tile([128, 1], FP32, tag="msum")
                            nc.scalar.activation(
                                p33[:], sim33[:], EXP, scale=SCALE, accum_out=msum[:]
                            )
                            # weighted values: sum_j p_j * mv_j  (d-major mv)
                            prod2 = pm.tile([128, KJD], BF16, tag="prod2")
                            mv3 = mv_t[:].rearrange("p (d j) -> p d j", j=KJ)
                            p_bc = p33[:].unsqueeze(1).broadcast_to([128, DH, KJ])
                            pr2v = prod2[:].rearrange("p (d j) -> p d j", j=KJ)
                            nc.gpsimd.tensor_mul(pr2v, mv3, p_bc)
                            memv = psm.tile([128, DH], FP32, tag="memv")
                            nc.vector.reduce_sum(memv[:], pr2v, axis=AX)
                            # ---- combine: o = pv*g/l + memv*(1-g)/m ----
                            linv = psm.tile([128, 1], FP32, tag="linv")
                            nc.vector.reciprocal(linv[:], pv[:, 64:65])
                            lg = psm.tile([128, 1], FP32, tag="lg")
                            nc.scalar.mul(lg[:], linv[:], gg_sb[:, h : h + 1])
                            minv = psm.tile([128, 1], FP32, tag="minv")
                            nc.vector.reciprocal(minv[:], msum[:])
                            mg = psm.tile([128, 1], FP32, tag="mg")
                            nc.scalar.mul(mg[:], minv[:], gg_sb[:, 2 + h : 3 + h])
                            osl = o2[:, h * 64 : (h + 1) * 64]
                            nc.vector.tensor_scalar(
                                osl, pv[:, 0:64], lg[:], None, op0=MULT,
                            )
                            tmp = psm.tile([128, DH], BF16, tag="tmp")
                            nc.vector.tensor_scalar(
                                tmp[:], memv[:], mg[:], None, op0=MULT
                            )
                            nc.vector.tensor_add(osl, osl, tmp[:])
                        # ---- output projection for this qi ----
                        otp = ps_c.tile([128, 128], BF16, tag="otp")
                        nc.tensor.transpose(otp[:], o2[:], iden_sb[:])
                        ot_sb = psm.tile([128, 128], BF16, tag="otsb")
                        nc.scalar.copy(ot_sb[:], otp[:])
                        for half in range(2):
                            op_ps = ps_c.tile([128, 512], FP32, tag="ops")
                            nc.tensor.matmul(
                                op_ps[:],
                                ot_sb[:],
                                wo_sb[:, half * 512 : (half + 1) * 512],
                                start=True,
                                stop=True,
                            )
                            out_sb = psm.tile([128, 512], BF16, tag="outsb")
                            nc.scalar.copy(out_sb[:], op_ps[:])
                            nc.sync.dma_start(
                                out_e[qi * 128 : (qi + 1) * 128,
                                      half * 512 : (half + 1) * 512],
                                out_sb[:],
                            )
    return nc


def _get_program():
    global _PROGRAM
    if _PROGRAM is None:
        _PROGRAM = _build_program()
    return _PROGRAM


def kernel(x, Wq, Wkv, Wo, bo, null_k, null_v, gate, mem_kv, mem_mask):
    x = np.asarray(x, dtype=np.float32)
    Wq = np.asarray(Wq, dtype=np.float32)
    Wkv = np.asarray(Wkv, dtype=np.float32)
    Wo = np.asarray(Wo, dtype=np.float32)
    bo = np.asarray(bo, dtype=np.float32)
    null_k = np.asarray(null_k, dtype=np.float32)
    null_v = np.asarray(null_v, dtype=np.float32)
    gate = np.asarray(gate, dtype=np.float32)
    mem_kv = np.asarray(mem_kv, dtype=np.float32)

    nc = _get_program()
    g = 1.0 / (1.0 + np.exp(-gate.reshape(H)))  # sigmoid, per head
    mem6 = mem_kv.reshape(B, H, N, K, 2, DH)

    # mem keys transposed for PE: [B, H, NT, 64, 4096], col = m*32 + j
    mkt_all = np.ascontiguousarray(
        mem6[..., 0, :]
        .reshape(B, H, NT, 128, K, DH)
        .transpose(0, 1, 2, 5, 3, 4)
        .reshape(B, H, NT, DH, 128 * K)
    ).astype(NPBF16)
    # mem values, d-major with null slot at j=0: [B, H, N, 64, 33]
    mv_all = np.empty((B, H, N, DH, KJ), dtype=NPBF16)
    mv_all[..., 0] = null_v.astype(NPBF16)
    mv_all[..., 1:] = np.swapaxes(mem6[..., 1, :], -1, -2).astype(NPBF16)

    xb = x.astype(NPBF16)
    Wqb = Wq.astype(NPBF16)
    Wkvb = Wkv.astype(NPBF16)
    Wob = Wo.astype(NPBF16)

    iden = np.eye(128, dtype=NPBF16)
    nk_col = null_k.reshape(DH, 1).astype(NPBF16)
    maskT = np.where(
        np.arange(128)[:, None] <= np.arange(128)[None, :], 0.0, NEG
    ).astype(np.float32)

    in_maps = []
    for c in range(8):
        b, hg = c // 4, c % 4
        h0 = hg * NH
        xT = np.ascontiguousarray(xb[b].T)
        wsb = np.empty((128, 2048), dtype=NPBF16)
        for d in range(8):
            wsb[:, d * 256 : d * 256 + 128] = Wqb[
                d * 128 : (d + 1) * 128, h0 * DH : (h0 + NH) * DH
            ]
            wsb[:, d * 256 + 128 : d * 256 + 256] = Wkvb[d * 128 : (d + 1) * 128, :]
        wo_c = np.ascontiguousarray(Wob[h0 * DH : (h0 + NH) * DH, :])
        mkt_c = mkt_all[b, h0 : h0 + NH]
        mv_c = mv_all[b, h0 : h0 + NH].reshape(NH, N, KJD)
        gg = np.zeros((128, 4), dtype=np.float32)
        gg[:, 0] = g[h0]
        gg[:, 1] = g[h0 + 1]
        gg[:, 2] = 1.0 - g[h0]
        gg[:, 3] = 1.0 - g[h0 + 1]
        in_maps.append(
            dict(
                xT=xT, wsb=wsb, wo=wo_c, mkt=mkt_c, mv=mv_c,
                nk=nk_col, gg=gg, mask=maskT, iden=iden,
            )
        )

    global _last_in_maps
    _last_in_maps = in_maps
    res = run_bass_kernel_spmd(nc, in_maps, list(range(8)))
    out = np.zeros((B, N, DIM), dtype=np.float32)
    for c in range(8):
        out[c // 4] += res.results[c]["out"].astype(np.float32)
    out += bo[None, None, :]
    return out
